# revision 14
# baseline (speedup 1.0000x reference)
"""Trainium2 Bass kernel for masked multi-head attention (B=8, S=1024, HID=1024, NH=16).

Computation (matches the torch/jax reference):
    q = query @ Wk.T + bk ; k = key @ Wk.T + bk ; v = value @ Wv.T + bv
    per head: scores = q k^T / 8, masked softmax over keys (mask zeroes masked
    positions), out = probs @ v.

Sharding: data-parallel over batch - batch element b runs on NeuronCore b.

v6 design notes:
  - all matmul operands are bf16 (psum fp32); fp8 was tried and rejected:
    weight-quantization error is coherent through the projection (Q error
    scales with |Q|, no sqrt(N) averaging) and blows the 2e-2 budget.
  - EVERY matmul uses the full 128x128 PE array (zero-padded per-head K^T,
    [V | ones | zeros] PV operand): half-array matmuls make the HAM
    activity monitor re-throttle the PE from 2.4 to 1.2 GHz.
  - one-group-lookahead pipeline: project block g, then run attention for
    block g-1's heads; K-proj before Q-proj and the Q eviction split
    ACT/DVE so psum-ring reuse never stalls the PE.
  - inputs are host-swizzled to partition-major [128, JC*n] so each DMA
    descriptor moves 4KB+ contiguous per partition instead of 2KB rows.
  - scores->exp->PV software-pipelined one kb-step deep; softmax denominator
    rides the PV matmul as a ones-column (psum row 64); reciprocal on DVE,
    partition-broadcast on GpSimd.
"""

import os
import sys
from contextlib import ExitStack

for _p in ("/opt/trn_rl_repo", "/root/.axon_site/_ro/trn_rl_repo"):
    if os.path.isdir(_p) and _p not in sys.path:
        sys.path.insert(0, _p)

import numpy as np
import ml_dtypes

from concourse import bacc, mybir, tile
from concourse.bass_utils import run_bass_kernel_spmd

B, S, HID, NH = 8, 1024, 1024, 16
HD = HID // NH  # 64
P = 128
JC = HID // P   # 8 contraction chunks of 128
OB = HID // P   # 8 output-column blocks
NEG = -1.0e30

F32 = mybir.dt.float32
BF16 = mybir.dt.bfloat16
AF = mybir.ActivationFunctionType
ALU = mybir.AluOpType
BDT = ml_dtypes.bfloat16

TRACE = os.environ.get("MHA_TRACE", "0") == "1"

_CACHE: dict = {}


def _ensure_axon_ntff_hook():
    """The agent image's antenv lacks axon_hooks; rebuild it from trn_boot's
    ctypes NTFF driver so trace=True can produce per-core profiles."""
    try:
        import antenv.axon_hooks  # noqa: F401

        return
    except ImportError:
        pass
    try:
        import types

        import antenv
        from trn_agent_boot.trn_boot import _ntff_profile_via_ctypes

        m = types.ModuleType("antenv.axon_hooks")
        m._hook = _ntff_profile_via_ctypes("/opt/axon/libaxon_pjrt.so")
        m.get_axon_ntff_profile_hook = lambda: m._hook
        m.set_axon_ntff_profile_hook = lambda h: setattr(m, "_hook", h)
        sys.modules["antenv.axon_hooks"] = m
        antenv.axon_hooks = m
    except Exception as e:  # pragma: no cover
        print(f"ntff hook shim unavailable: {e}", file=sys.stderr)


def _segs(n):
    """Split [0, n) into <=512 pieces aligned to the 512-col psum banks."""
    return [(a, min(a + 512, n)) for a in range(0, n, 512)]


def _build(KB: int):
    """Build the SPMD program for compacted key length KC = KB*128."""
    KC = KB * P
    nc = bacc.Bacc("TRN2", target_bir_lowering=False, debug=False)
    names = {}

    with tile.TileContext(nc) as tc, ExitStack() as ctx:
        dram = ctx.enter_context(tc.tile_pool(name="dram", bufs=1, space="DRAM"))

        def din(nm, shape, dt=BF16):
            t = dram.tile(shape, dt, kind="ExternalInput", name=nm, uniquify=False)
            names[nm] = t.name
            return t

        # partition-major swizzled inputs: [128, JC, n]
        qT_d = din("qT", [P, JC, S])
        kT_d = din("kT", [P, JC, KC])
        vT_d = din("vT", [P, JC, KC])
        WkT_d = din("WkT", [P, JC, HID])
        WvT_d = din("WvT", [P, JC, HID])
        bkc_d = din("bkc", [P, OB], F32)
        bvb_d = din("bvb", [P, HID], F32)
        mkc_d = din("mkc", [P, KB], F32)
        hm_d = din("hm", [P, 2], F32)
        outT_d = dram.tile(
            [HID, S], BF16, kind="ExternalOutput", name="outT", uniquify=False
        )
        names["out"] = outT_d.name

        res = ctx.enter_context(tc.tile_pool(name="res", bufs=1))
        QT = res.tile([P, OB, S], BF16, tag="QT")         # Q^T  [o, s]
        KTz = res.tile([P, NH, KC], BF16, tag="KTz")      # per-head padded K^T
        Vx = res.tile([P, KB, NH, P], BF16, tag="Vx")     # [s(k), kb, head, d|1|0]
        bkc = res.tile([P, OB], F32, tag="bkc")
        bvb = res.tile([P, HID], F32, tag="bvb")
        mkc = res.tile([P, KB], F32, tag="mkc")
        hm = res.tile([P, 2], F32, tag="hm")

        # input staging
        qTt = res.tile([P, JC, S], BF16, tag="qTt")
        kTt = res.tile([P, JC, KC], BF16, tag="kTt")
        vTt = res.tile([P, JC, KC], BF16, tag="vTt")
        WkTt = res.tile([P, JC, HID], BF16, tag="WkTt")
        WvTt = res.tile([P, JC, HID], BF16, tag="WvTt")

        psS = ctx.enter_context(tc.tile_pool(name="psS", bufs=2, space="PSUM"))
        psO = ctx.enter_context(tc.tile_pool(name="psO", bufs=2, space="PSUM"))

        ptp = ctx.enter_context(tc.tile_pool(name="ptp", bufs=4))
        outp = ctx.enter_context(tc.tile_pool(name="outp", bufs=3))
        bcp = ctx.enter_context(tc.tile_pool(name="bcp", bufs=3))
        smalls = ctx.enter_context(tc.tile_pool(name="smalls", bufs=3))

        # PE warm-up: dummy matmuls with no data deps run during the initial
        # DMA fill so the HAM clock-gate reaches 8/8 before real work.
        wu = res.tile([P, 512], BF16, tag="wu")
        nc.vector.memset(wu[:], 0.0)
        wu_sink = dram.tile(
            [1, 1], F32, kind="ExternalOutput", name="wu_sink", uniquify=False
        )
        wps = psS.tile([P, 512], F32, tag="S", name="wu_ps")
        NWU = 20
        for i in range(NWU):
            nc.tensor.matmul(
                wps[:], wu[:, 0:128], wu[:], start=(i == 0), stop=(i == NWU - 1)
            )
        wu_sb = res.tile([1, 1], F32, tag="wu_sb")
        nc.vector.tensor_copy(wu_sb[:], wps[0:1, 0:1])
        nc.sync.dma_start(wu_sink[:], wu_sb[:])
        # dep-free filler matmuls: interleaved into the DMA-paced V phase to
        # plug PE idle gaps (keeps the HAM clock-gate warm, costs nothing --
        # the PE queue is in-order so they run exactly when V data is late)
        fps = psO.tile([P, 512], F32, tag="O", name="fill_ps")

        def filler(n):
            for _ in range(n):
                nc.tensor.matmul(fps[:], wu[:, 0:128], wu[:], start=True, stop=True)

        onef = res.tile([P, 1], F32, tag="onef")
        nc.vector.memset(onef[:], 1.0)
        # DMAs in consumption order, two chunks per transfer (4KB+ per
        # partition descriptor): (vT,WvT) pairs, (kT,WkT) pairs, qT.
        for c in range(0, JC, 2):
            nc.sync.dma_start(vTt[:, c : c + 2], vT_d[:, c : c + 2])
            nc.sync.dma_start(WvTt[:, c : c + 2], WvT_d[:, c : c + 2])
        for c in range(0, JC, 2):
            nc.scalar.dma_start(kTt[:, c : c + 2], kT_d[:, c : c + 2])
            nc.scalar.dma_start(WkTt[:, c : c + 2], WkT_d[:, c : c + 2])
        for c in range(0, JC, 2):
            nc.sync.dma_start(qTt[:, c : c + 2], qT_d[:, c : c + 2])
        nc.sync.dma_start(bkc[:], bkc_d[:])
        nc.sync.dma_start(bvb[:], bvb_d[:])
        nc.sync.dma_start(mkc[:], mkc_d[:])
        nc.sync.dma_start(hm[:], hm_d[:])
        # V padding: zero the pad columns once, then the ones-column of each
        # head slot (col 64); the V-proj evictions fill cols 0..63.
        nc.vector.memset(Vx[:, :, :, HD + 1 :], 0.0)
        nc.vector.tensor_copy(
            Vx[:, :, :, HD], onef[:].broadcast_to((P, KB, NH))
        )

        # ---------------- phase V: V = value @ Wv^T + bv (natural [s, o]) ---
        # sb-blocks run 3-wide (c-outer) so the PE consumption rate of the
        # (vT, WvT) chunk pairs matches the DMA delivery rate.
        for sb0 in (0, 3):
            sbs = list(range(sb0, min(sb0 + 3, KB)))
            pss = {
                sb: psS.tile([P, HID], F32, tag="S", name=f"psv{sb}") for sb in sbs
            }
            for c in range(JC):
                for sb in sbs:
                    lhsT = vTt[:, c, sb * P : (sb + 1) * P]
                    for a, b in _segs(HID):
                        nc.tensor.matmul(
                            pss[sb][:, a:b], lhsT, WvTt[:, c, a:b],
                            start=(c == 0), stop=(c == JC - 1),
                        )
                filler(5)
            for sb in sbs:
                # evict with +bv into the padded layout (DVE; idle here)
                nc.vector.tensor_add(
                    Vx[:, sb, :, 0:HD],
                    pss[sb][:].rearrange("p (h c) -> p h c", c=HD),
                    bvb[:].rearrange("p (h c) -> p h c", c=HD),
                )

        # ---- one-group-lookahead pipeline: project block g, then run the ----
        # ---- attention for block g-1's heads (evictions get a full group ----
        # ---- of slack before the scores matmuls need them)               ----
        def issue_proj(g):
            # K-proj block g (first: its psum buf is reused by S(2(g-1),0),
            # which runs a whole Q-proj later)
            psk = psS.tile([P, KC], F32, tag="S", name=f"psk{g}")
            for c in range(JC):
                lhsT = WkTt[:, c, g * P : (g + 1) * P]
                for a, b in _segs(KC):
                    nc.tensor.matmul(
                        psk[:, a:b], lhsT, kTt[:, c, a:b],
                        start=(c == 0), stop=(c == JC - 1),
                    )
            # padded evictions: KTz[h] = (psk + bk) * head-half row mask
            for half in range(2):
                nc.vector.tensor_scalar(
                    KTz[:, 2 * g + half, :], psk[:],
                    bkc[:, g : g + 1], hm[:, half : half + 1],
                    ALU.add, ALU.mult,
                )
            # Q-proj block g
            psq = psS.tile([P, S], F32, tag="S", name=f"psq{g}")
            for c in range(JC):
                lhsT = WkTt[:, c, g * P : (g + 1) * P]
                for a, b in _segs(S):
                    nc.tensor.matmul(
                        psq[:, a:b], lhsT, qTt[:, c, a:b],
                        start=(c == 0), stop=(c == JC - 1),
                    )
            # eviction split ACT/DVE so the last psum reader finishes fast
            nc.scalar.activation(
                QT[:, g, 0:512], psq[:, 0:512], AF.Identity, bias=bkc[:, g : g + 1]
            )
            nc.vector.tensor_scalar_add(
                QT[:, g, 512:S], psq[:, 512:S], bkc[:, g : g + 1]
            )

        def issue_attn(g):
            for half in range(2):
                h = 2 * g + half
                Ops = psO.tile([P, S], F32, tag="O", name=f"O{h}")
                # software-pipelined S/exp/PV: S0 S1 PV0 S2 PV1 S3 PV2 S4 PV3 PV4
                PTs = [None] * KB

                def issue_S(kb):
                    Sps = psS.tile([P, S], F32, tag="S", name=f"S{h}_{kb}")
                    lhsT = KTz[:, h, kb * P : (kb + 1) * P]
                    for a, b in _segs(S):
                        nc.tensor.matmul(
                            Sps[:, a:b], lhsT, QT[:, g, a:b],
                            start=True, stop=True,
                        )
                    PT = ptp.tile([P, S], BF16, tag="PT", name=f"PT{h}_{kb}")
                    nc.scalar.activation(
                        PT[:], Sps[:], AF.Exp, bias=mkc[:, kb : kb + 1], scale=0.125
                    )
                    PTs[kb] = PT

                def issue_PV(kb):
                    Vl = Vx[:, kb, h, :]
                    for a, b in _segs(S):
                        nc.tensor.matmul(
                            Ops[:, a:b], Vl, PTs[kb][:, a:b],
                            start=(kb == 0), stop=(kb == KB - 1),
                        )

                issue_S(0)
                for kb in range(1, KB):
                    issue_S(kb)
                    issue_PV(kb - 1)
                issue_PV(KB - 1)

                # normalize: denom row 64 -> recip (DVE) -> partition
                # broadcast (GpSimd) -> multiply (DVE) -> DMA out.
                # The final head runs the chain per 512-col half so the
                # end-of-kernel drain overlaps instead of serializing.
                halves = _segs(S) if h == NH - 1 else [(0, S)]
                On = outp.tile([HD, S], BF16, tag="On", name=f"On{h}")
                for a, b in halves:
                    w = b - a
                    rden = smalls.tile([1, S], F32, tag="rden", name=f"rden{h}_{a}")
                    nc.vector.tensor_copy(rden[:, 0:w], Ops[HD : HD + 1, a:b])
                    rrec = smalls.tile([1, S], F32, tag="rrec", name=f"rrec{h}_{a}")
                    nc.vector.reciprocal_approx_fast(rrec[:, 0:w], rden[:, 0:w])
                    bcb = bcp.tile([HD, S], F32, tag="bcb", name=f"bcb{h}_{a}")
                    nc.gpsimd.partition_broadcast(bcb[:, 0:w], rrec[:, 0:w])
                    nc.vector.tensor_mul(On[:, a:b], Ops[0:HD, a:b], bcb[:, 0:w])
                    nc.sync.dma_start(outT_d[h * HD : (h + 1) * HD, a:b], On[:, a:b])

        for g in range(OB + 1):
            if g < OB:
                issue_proj(g)
            if g >= 1:
                issue_attn(g - 1)

    nc.compile()
    return nc, names


def _swz(xT):
    """[1024(j), n] -> partition-major [128, JC, n] bf16:
    element (p, c, :) = xT[c*128 + p, :]  (4KB+ contiguous per partition)."""
    n = xT.shape[1]
    return np.ascontiguousarray(
        xT.reshape(JC, P, n).transpose(1, 0, 2)
    ).astype(BDT)


def _prep(query, key, value, attention_mask, Wk, bk, Wv, bv):
    """Host-side sharding + layout prep. Returns (KB, in_maps, empty_batches)."""
    query = np.ascontiguousarray(np.asarray(query, dtype=np.float32))
    key = np.ascontiguousarray(np.asarray(key, dtype=np.float32))
    value = np.ascontiguousarray(np.asarray(value, dtype=np.float32))
    mask = np.asarray(attention_mask).reshape(B, S) != 0
    Wk = np.asarray(Wk, dtype=np.float32)
    bk = np.asarray(bk, dtype=np.float32)
    Wv = np.asarray(Wv, dtype=np.float32)
    bv = np.asarray(bv, dtype=np.float32)

    idxs, counts = [], []
    for b in range(B):
        ix = np.flatnonzero(mask[b])
        idxs.append(ix)
        counts.append(len(ix))
    KC = max(int(np.ceil(max(max(counts), 1) / P)) * P, P)
    KB = KC // P

    WkT8 = _swz(np.ascontiguousarray(Wk.T))
    WvT8 = _swz(np.ascontiguousarray(Wv.T))
    bkc = np.ascontiguousarray(bk.reshape(OB, P).T)         # [128, 8]
    bvb = np.ascontiguousarray(np.broadcast_to(bv, (P, HID)))
    hm = np.zeros((P, 2), dtype=np.float32)
    hm[0:HD, 0] = 1.0
    hm[HD:P, 1] = 1.0

    in_maps = []
    empty = []
    for b in range(B):
        n = counts[b]
        if n == 0:
            empty.append(b)
        ix = idxs[b] if n > 0 else np.array([0])
        pad = np.concatenate([ix, np.full(KC - len(ix), ix[0], dtype=ix.dtype)])
        mb = np.zeros(KC, dtype=np.float32)
        mb[n:] = NEG
        in_maps.append(
            {
                "qT": _swz(query[b].T),
                "kT": _swz(key[b].T[:, pad]),
                "vT": _swz(value[b].T[:, pad]),
                "WkT": WkT8,
                "WvT": WvT8,
                "bkc": bkc,
                "bvb": bvb,
                "mkc": np.ascontiguousarray(mb.reshape(KB, P).T),
                "hm": hm,
            }
        )
    return KB, in_maps, empty


def kernel(key, value, query, attention_mask, Wk, bk, Wv, bv):
    KB, in_maps, empty = _prep(query, key, value, attention_mask, Wk, bk, Wv, bv)

    if KB not in _CACHE:
        _CACHE[KB] = _build(KB)
    nc, names = _CACHE[KB]

    # remap host arrays onto the (possibly uniquified) dram tensor names
    mapped = [
        {names[k]: v for k, v in m.items()} for m in in_maps
    ]
    if TRACE:
        _ensure_axon_ntff_hook()
    res = run_bass_kernel_spmd(nc, mapped, list(range(B)), trace=TRACE)
    if TRACE and res.exec_time_ns is not None:
        print(f"HW exec time: {res.exec_time_ns} ns")

    out = np.empty((B, S, HID), dtype=np.float32)
    for b in range(B):
        out[b] = res.results[b][names["out"]].astype(np.float32).T
    for b in empty:
        out[b] = 0.0
    return out


# revision 18
# speedup vs baseline: 1.0278x; 1.0278x over previous
"""Trainium2 Bass kernel for masked multi-head attention (B=8, S=1024, HID=1024, NH=16).

Computation (matches the torch/jax reference):
    q = query @ Wk.T + bk ; k = key @ Wk.T + bk ; v = value @ Wv.T + bv
    per head: scores = q k^T / 8, masked softmax over keys (mask zeroes masked
    positions), out = probs @ v.

Sharding: data-parallel over batch - batch element b runs on NeuronCore b.

v6 design notes:
  - all matmul operands are bf16 (psum fp32); fp8 was tried and rejected:
    weight-quantization error is coherent through the projection (Q error
    scales with |Q|, no sqrt(N) averaging) and blows the 2e-2 budget.
  - EVERY matmul uses the full 128x128 PE array (zero-padded per-head K^T,
    [V | ones | zeros] PV operand): half-array matmuls make the HAM
    activity monitor re-throttle the PE from 2.4 to 1.2 GHz.
  - one-group-lookahead pipeline: project block g, then run attention for
    block g-1's heads; K-proj before Q-proj and the Q eviction split
    ACT/DVE so psum-ring reuse never stalls the PE.
  - inputs are host-swizzled to partition-major [128, JC*n] so each DMA
    descriptor moves 4KB+ contiguous per partition instead of 2KB rows.
  - scores->exp->PV software-pipelined one kb-step deep; softmax denominator
    rides the PV matmul as a ones-column (psum row 64); reciprocal on DVE,
    partition-broadcast on GpSimd.
"""

import os
import sys
from contextlib import ExitStack

for _p in ("/opt/trn_rl_repo", "/root/.axon_site/_ro/trn_rl_repo"):
    if os.path.isdir(_p) and _p not in sys.path:
        sys.path.insert(0, _p)

import numpy as np
import ml_dtypes

from concourse import bacc, mybir, tile
from concourse.bass_utils import run_bass_kernel_spmd

B, S, HID, NH = 8, 1024, 1024, 16
HD = HID // NH  # 64
P = 128
JC = HID // P   # 8 contraction chunks of 128
OB = HID // P   # 8 output-column blocks
NEG = -1.0e30

F32 = mybir.dt.float32
BF16 = mybir.dt.bfloat16
AF = mybir.ActivationFunctionType
ALU = mybir.AluOpType
BDT = ml_dtypes.bfloat16

TRACE = os.environ.get("MHA_TRACE", "0") == "1"

_CACHE: dict = {}


def _ensure_axon_ntff_hook():
    """The agent image's antenv lacks axon_hooks; rebuild it from trn_boot's
    ctypes NTFF driver so trace=True can produce per-core profiles."""
    try:
        import antenv.axon_hooks  # noqa: F401

        return
    except ImportError:
        pass
    try:
        import types

        import antenv
        from trn_agent_boot.trn_boot import _ntff_profile_via_ctypes

        m = types.ModuleType("antenv.axon_hooks")
        m._hook = _ntff_profile_via_ctypes("/opt/axon/libaxon_pjrt.so")
        m.get_axon_ntff_profile_hook = lambda: m._hook
        m.set_axon_ntff_profile_hook = lambda h: setattr(m, "_hook", h)
        sys.modules["antenv.axon_hooks"] = m
        antenv.axon_hooks = m
    except Exception as e:  # pragma: no cover
        print(f"ntff hook shim unavailable: {e}", file=sys.stderr)


def _segs(n):
    """Split [0, n) into <=512 pieces aligned to the 512-col psum banks."""
    return [(a, min(a + 512, n)) for a in range(0, n, 512)]


def _build(KB: int):
    """Build the SPMD program for compacted key length KC = KB*128."""
    KC = KB * P
    nc = bacc.Bacc("TRN2", target_bir_lowering=False, debug=False)
    names = {}

    with tile.TileContext(nc) as tc, ExitStack() as ctx:
        dram = ctx.enter_context(tc.tile_pool(name="dram", bufs=1, space="DRAM"))

        def din(nm, shape, dt=BF16):
            t = dram.tile(shape, dt, kind="ExternalInput", name=nm, uniquify=False)
            names[nm] = t.name
            return t

        # partition-major swizzled inputs: [128, JC, n]
        qT_d = din("qT", [P, JC, S])
        kT_d = din("kT", [P, JC, KC])
        vT_d = din("vT", [P, JC, KC])
        WkT_d = din("WkT", [P, JC, HID])
        WvT_d = din("WvT", [P, JC, HID])
        bkc_d = din("bkc", [P, OB], F32)
        bvb_d = din("bvb", [P, HID], F32)
        mkc_d = din("mkc", [P, KB], F32)
        hm_d = din("hm", [P, 2], F32)
        outT_d = dram.tile(
            [HID, S], BF16, kind="ExternalOutput", name="outT", uniquify=False
        )
        names["out"] = outT_d.name

        res = ctx.enter_context(tc.tile_pool(name="res", bufs=1))
        QT = res.tile([P, OB, S], BF16, tag="QT")         # Q^T  [o, s]
        KTz = res.tile([P, NH, KC], BF16, tag="KTz")      # per-head padded K^T
        Vx = res.tile([P, KB, NH, P], BF16, tag="Vx")     # [s(k), kb, head, d|1|0]
        bkc = res.tile([P, OB], F32, tag="bkc")
        bvb = res.tile([P, HID], F32, tag="bvb")
        mkc = res.tile([P, KB], F32, tag="mkc")
        hm = res.tile([P, 2], F32, tag="hm")

        # input staging
        qTt = res.tile([P, JC, S], BF16, tag="qTt")
        kTt = res.tile([P, JC, KC], BF16, tag="kTt")
        vTt = res.tile([P, JC, KC], BF16, tag="vTt")
        WkTt = res.tile([P, JC, HID], BF16, tag="WkTt")
        WvTt = res.tile([P, JC, HID], BF16, tag="WvTt")

        psS = ctx.enter_context(tc.tile_pool(name="psS", bufs=2, space="PSUM"))
        psO = ctx.enter_context(tc.tile_pool(name="psO", bufs=2, space="PSUM"))

        ptp = ctx.enter_context(tc.tile_pool(name="ptp", bufs=4))
        outp = ctx.enter_context(tc.tile_pool(name="outp", bufs=3))
        bcp = ctx.enter_context(tc.tile_pool(name="bcp", bufs=3))
        smalls = ctx.enter_context(tc.tile_pool(name="smalls", bufs=3))

        # PE warm-up: dummy matmuls with no data deps run during the initial
        # DMA fill so the HAM clock-gate reaches 8/8 before real work.
        wu = res.tile([P, 512], BF16, tag="wu")
        nc.vector.memset(wu[:], 0.0)
        wu_sink = dram.tile(
            [1, 1], F32, kind="ExternalOutput", name="wu_sink", uniquify=False
        )
        wps = psS.tile([P, 512], F32, tag="S", name="wu_ps")
        NWU = 20
        for i in range(NWU):
            nc.tensor.matmul(
                wps[:], wu[:, 0:128], wu[:], start=(i == 0), stop=(i == NWU - 1)
            )
        wu_sb = res.tile([1, 1], F32, tag="wu_sb")
        nc.vector.tensor_copy(wu_sb[:], wps[0:1, 0:1])
        nc.sync.dma_start(wu_sink[:], wu_sb[:])
        # dep-free filler matmuls: inserted only where the trace shows the
        # DMA-paced first V pass starving the PE (keeps HAM warm for free)
        fps = psO.tile([P, 512], F32, tag="O", name="fill_ps")

        def filler(n):
            for _ in range(n):
                nc.tensor.matmul(fps[:], wu[:, 0:128], wu[:], start=True, stop=True)

        onef = res.tile([P, 1], F32, tag="onef")
        nc.vector.memset(onef[:], 1.0)
        # DMAs in consumption order, two chunks per transfer (4KB+ per
        # partition descriptor): (vT,WvT) pairs, (kT,WkT) pairs, qT.
        for c in range(0, JC, 2):
            nc.sync.dma_start(vTt[:, c : c + 2], vT_d[:, c : c + 2])
            nc.sync.dma_start(WvTt[:, c : c + 2], WvT_d[:, c : c + 2])
        for c in range(0, JC, 2):
            nc.scalar.dma_start(kTt[:, c : c + 2], kT_d[:, c : c + 2])
            nc.scalar.dma_start(WkTt[:, c : c + 2], WkT_d[:, c : c + 2])
        for c in range(0, JC, 2):
            nc.sync.dma_start(qTt[:, c : c + 2], qT_d[:, c : c + 2])
        nc.sync.dma_start(bkc[:], bkc_d[:])
        nc.sync.dma_start(bvb[:], bvb_d[:])
        nc.sync.dma_start(mkc[:], mkc_d[:])
        nc.sync.dma_start(hm[:], hm_d[:])
        # V padding: zero the pad columns once, then the ones-column of each
        # head slot (col 64); the V-proj evictions fill cols 0..63.
        nc.vector.memset(Vx[:, :, :, HD + 1 :], 0.0)
        nc.vector.tensor_copy(
            Vx[:, :, :, HD], onef[:].broadcast_to((P, KB, NH))
        )

        # ---------------- phase V: V = value @ Wv^T + bv (natural [s, o]) ---
        # sb-blocks run 3-wide (c-outer) so the PE consumption rate of the
        # (vT, WvT) chunk pairs matches the DMA delivery rate.
        for sb0 in (0, 3):
            sbs = list(range(sb0, min(sb0 + 3, KB)))
            pss = {
                sb: psS.tile([P, HID], F32, tag="S", name=f"psv{sb}") for sb in sbs
            }
            for c in range(JC):
                for sb in sbs:
                    lhsT = vTt[:, c, sb * P : (sb + 1) * P]
                    for a, b in _segs(HID):
                        nc.tensor.matmul(
                            pss[sb][:, a:b], lhsT, WvTt[:, c, a:b],
                            start=(c == 0), stop=(c == JC - 1),
                        )
                if sb0 == 0 and c <= 5:
                    filler(4)
            for sb in sbs:
                # evict with +bv into the padded layout (DVE; idle here)
                nc.vector.tensor_add(
                    Vx[:, sb, :, 0:HD],
                    pss[sb][:].rearrange("p (h c) -> p h c", c=HD),
                    bvb[:].rearrange("p (h c) -> p h c", c=HD),
                )

        # ---- one-group-lookahead pipeline: project block g, then run the ----
        # ---- attention for block g-1's heads (evictions get a full group ----
        # ---- of slack before the scores matmuls need them)               ----
        def issue_proj(g):
            # K-proj block g (first: its psum buf is reused by S(2(g-1),0),
            # which runs a whole Q-proj later)
            psk = psS.tile([P, KC], F32, tag="S", name=f"psk{g}")
            for c in range(JC):
                lhsT = WkTt[:, c, g * P : (g + 1) * P]
                for a, b in _segs(KC):
                    nc.tensor.matmul(
                        psk[:, a:b], lhsT, kTt[:, c, a:b],
                        start=(c == 0), stop=(c == JC - 1),
                    )
            # padded evictions: KTz[h] = (psk + bk) * head-half row mask
            for half in range(2):
                nc.vector.tensor_scalar(
                    KTz[:, 2 * g + half, :], psk[:],
                    bkc[:, g : g + 1], hm[:, half : half + 1],
                    ALU.add, ALU.mult,
                )
            # Q-proj block g
            psq = psS.tile([P, S], F32, tag="S", name=f"psq{g}")
            for c in range(JC):
                lhsT = WkTt[:, c, g * P : (g + 1) * P]
                for a, b in _segs(S):
                    nc.tensor.matmul(
                        psq[:, a:b], lhsT, qTt[:, c, a:b],
                        start=(c == 0), stop=(c == JC - 1),
                    )
            # eviction split ACT/DVE so the last psum reader finishes fast
            nc.scalar.activation(
                QT[:, g, 0:512], psq[:, 0:512], AF.Identity, bias=bkc[:, g : g + 1]
            )
            nc.vector.tensor_scalar_add(
                QT[:, g, 512:S], psq[:, 512:S], bkc[:, g : g + 1]
            )

        def issue_attn(g):
            for half in range(2):
                h = 2 * g + half
                Ops = psO.tile([P, S], F32, tag="O", name=f"O{h}")
                # software-pipelined S/exp/PV: S0 S1 PV0 S2 PV1 S3 PV2 S4 PV3 PV4
                PTs = [None] * KB

                def issue_S(kb):
                    Sps = psS.tile([P, S], F32, tag="S", name=f"S{h}_{kb}")
                    lhsT = KTz[:, h, kb * P : (kb + 1) * P]
                    for a, b in _segs(S):
                        nc.tensor.matmul(
                            Sps[:, a:b], lhsT, QT[:, g, a:b],
                            start=True, stop=True,
                        )
                    PT = ptp.tile([P, S], BF16, tag="PT", name=f"PT{h}_{kb}")
                    nc.scalar.activation(
                        PT[:], Sps[:], AF.Exp, bias=mkc[:, kb : kb + 1], scale=0.125
                    )
                    PTs[kb] = PT

                def issue_PV(kb):
                    Vl = Vx[:, kb, h, :]
                    for a, b in _segs(S):
                        nc.tensor.matmul(
                            Ops[:, a:b], Vl, PTs[kb][:, a:b],
                            start=(kb == 0), stop=(kb == KB - 1),
                        )

                issue_S(0)
                for kb in range(1, KB):
                    issue_S(kb)
                    issue_PV(kb - 1)
                issue_PV(KB - 1)

                # normalize: denom row 64 -> recip (DVE) -> partition
                # broadcast (GpSimd) -> multiply (DVE) -> DMA out.
                # The final head runs the chain per 512-col half so the
                # end-of-kernel drain overlaps instead of serializing.
                halves = _segs(S) if h == NH - 1 else [(0, S)]
                On = outp.tile([HD, S], BF16, tag="On", name=f"On{h}")
                for a, b in halves:
                    w = b - a
                    rden = smalls.tile([1, S], F32, tag="rden", name=f"rden{h}_{a}")
                    nc.vector.tensor_copy(rden[:, 0:w], Ops[HD : HD + 1, a:b])
                    rrec = smalls.tile([1, S], F32, tag="rrec", name=f"rrec{h}_{a}")
                    nc.vector.reciprocal_approx_fast(rrec[:, 0:w], rden[:, 0:w])
                    bcb = bcp.tile([HD, S], F32, tag="bcb", name=f"bcb{h}_{a}")
                    nc.gpsimd.partition_broadcast(bcb[:, 0:w], rrec[:, 0:w])
                    nc.vector.tensor_mul(On[:, a:b], Ops[0:HD, a:b], bcb[:, 0:w])
                    nc.sync.dma_start(outT_d[h * HD : (h + 1) * HD, a:b], On[:, a:b])

        for g in range(OB + 1):
            if g < OB:
                issue_proj(g)
            if g >= 1:
                issue_attn(g - 1)

    nc.compile()
    return nc, names


def _swz(xT):
    """[1024(j), n] -> partition-major [128, JC, n] bf16:
    element (p, c, :) = xT[c*128 + p, :]  (4KB+ contiguous per partition)."""
    n = xT.shape[1]
    return np.ascontiguousarray(
        xT.reshape(JC, P, n).transpose(1, 0, 2)
    ).astype(BDT)


def _prep(query, key, value, attention_mask, Wk, bk, Wv, bv):
    """Host-side sharding + layout prep. Returns (KB, in_maps, empty_batches)."""
    query = np.ascontiguousarray(np.asarray(query, dtype=np.float32))
    key = np.ascontiguousarray(np.asarray(key, dtype=np.float32))
    value = np.ascontiguousarray(np.asarray(value, dtype=np.float32))
    mask = np.asarray(attention_mask).reshape(B, S) != 0
    Wk = np.asarray(Wk, dtype=np.float32)
    bk = np.asarray(bk, dtype=np.float32)
    Wv = np.asarray(Wv, dtype=np.float32)
    bv = np.asarray(bv, dtype=np.float32)

    idxs, counts = [], []
    for b in range(B):
        ix = np.flatnonzero(mask[b])
        idxs.append(ix)
        counts.append(len(ix))
    KC = max(int(np.ceil(max(max(counts), 1) / P)) * P, P)
    KB = KC // P

    WkT8 = _swz(np.ascontiguousarray(Wk.T))
    WvT8 = _swz(np.ascontiguousarray(Wv.T))
    bkc = np.ascontiguousarray(bk.reshape(OB, P).T)         # [128, 8]
    bvb = np.ascontiguousarray(np.broadcast_to(bv, (P, HID)))
    hm = np.zeros((P, 2), dtype=np.float32)
    hm[0:HD, 0] = 1.0
    hm[HD:P, 1] = 1.0

    in_maps = []
    empty = []
    for b in range(B):
        n = counts[b]
        if n == 0:
            empty.append(b)
        ix = idxs[b] if n > 0 else np.array([0])
        pad = np.concatenate([ix, np.full(KC - len(ix), ix[0], dtype=ix.dtype)])
        mb = np.zeros(KC, dtype=np.float32)
        mb[n:] = NEG
        in_maps.append(
            {
                "qT": _swz(query[b].T),
                "kT": _swz(key[b].T[:, pad]),
                "vT": _swz(value[b].T[:, pad]),
                "WkT": WkT8,
                "WvT": WvT8,
                "bkc": bkc,
                "bvb": bvb,
                "mkc": np.ascontiguousarray(mb.reshape(KB, P).T),
                "hm": hm,
            }
        )
    return KB, in_maps, empty


def kernel(key, value, query, attention_mask, Wk, bk, Wv, bv):
    KB, in_maps, empty = _prep(query, key, value, attention_mask, Wk, bk, Wv, bv)

    if KB not in _CACHE:
        _CACHE[KB] = _build(KB)
    nc, names = _CACHE[KB]

    # remap host arrays onto the (possibly uniquified) dram tensor names
    mapped = [
        {names[k]: v for k, v in m.items()} for m in in_maps
    ]
    if TRACE:
        _ensure_axon_ntff_hook()
    res = run_bass_kernel_spmd(nc, mapped, list(range(B)), trace=TRACE)
    if TRACE and res.exec_time_ns is not None:
        print(f"HW exec time: {res.exec_time_ns} ns")

    out = np.empty((B, S, HID), dtype=np.float32)
    for b in range(B):
        out[b] = res.results[b][names["out"]].astype(np.float32).T
    for b in empty:
        out[b] = 0.0
    return out
